# revision 1
# baseline (speedup 1.0000x reference)
"""Trainium2 Bass kernel for GQA attention with RoPE (causal), dp2 x tp4
across 8 NeuronCores.

Reference computation (all fp32):
  q = (x @ wq.T)  -> [B,S,16,128], k/v = (x @ wk/wv.T) -> [B,S,4,128]
  q,k roped (interleaved-pair rotation); repeat_kv(4);
  causal softmax(qk/sqrt(128)) @ v; out = attn @ wo.T

Sharding: core i handles batch i//4 and q heads {4g..4g+3}, g = i%4, plus
kv head g (exactly the kv head those q heads attend to). wq/wk/wv are
column-sharded, wo row-sharded; the all-reduce over the 4 partial wo
outputs per batch happens on the host.

Precision strategy (rel-err budget 2e-2; this lands ~4e-3):
 - Projections and the wo matmul run in compensated fp8-e4m3 DoubleRow
   (x = xh+xl, w = wh+wl, drop xl*wl): 3 passes of 256-deep contraction
   = 0.75x the fp32r cycle count at ~0.15% error. Operands are pre-scaled
   (x*8, w*64) into e4m3's normal range; the 1/512 rescale is folded into
   the PSUM drain (q/k/v) or the host-side sum (y).
 - Attention internals are bf16 (same PE rate as fp32r, 2-4x DVE rate,
   half the DMA bytes). Softmax has no max-subtraction (|scores|<~5).
 - o is re-quantized to fp8 hi/lo for the wo matmul; the *8 scale is
   folded into the softmax ones-vector (ones = 1/8 so 1/rowsum carries 8).

Layout notes: head_dim rows of wq/wk are host-permuted so RoPE pairs are
de-interleaved (real parts rows 0..63, imag rows 64..127 per head); scores
are invariant since q and k get the same permutation. RoPE runs on bf16
SBUF tiles: ACT drains PSUM->bf16, a SBUF->SBUF DMA builds the
half-swapped copy, and 4 DVE ops (all operand pairs base-aligned) finish
the rotation.

Rowsums ride the PE as ones-matmuls accumulating into one PSUM bank at
partition offsets 32h (tile_position), one bank for all 4 heads.
"""

import itertools
import math
import sys
from contextlib import ExitStack

import numpy as np

if "/opt/trn_rl_repo" not in sys.path:
    sys.path.insert(0, "/opt/trn_rl_repo")

B = 2
S = 2048
D = 2048
N_HEADS = 16
N_KV_HEADS = 4
HEAD_DIM = 128
N_CORES = 8
HPC = 4  # q heads per core
SC = 512  # sequence chunk
NCH = S // SC  # 4
NKO = D // 256  # 8 DoubleRow plane-pairs over the contraction
NW = 6  # proj col-blocks per chunk: q0..q3, k, v
SCALE = 1.0 / math.sqrt(HEAD_DIM)
SX = 8.0  # x pre-scale for e4m3
SW = 64.0  # w / wo pre-scale
SO = 8.0  # o pre-scale (folded into ones = 1/SO)
PSC = SX * SW  # 512: proj psum scale

_CACHE = {}


def _build_module():
    import concourse.tile as tile
    from concourse import bacc, mybir

    f32 = mybir.dt.float32
    bf16 = mybir.dt.bfloat16
    e4 = mybir.dt.float8e4
    DR = mybir.MatmulPerfMode.DoubleRow
    Exp = mybir.ActivationFunctionType.Exp

    nc = bacc.Bacc(
        "TRN2",
        target_bir_lowering=False,
        debug=False,
        enable_asserts=False,
        num_devices=N_CORES,
    )
    # packed [p, ko, two, *]: contraction index d = 256*ko + 128*two + p
    xh = nc.dram_tensor("xh", [128, NKO, 2, S], e4, kind="ExternalInput").ap()
    xl = nc.dram_tensor("xl", [128, NKO, 2, S], e4, kind="ExternalInput").ap()
    wh = nc.dram_tensor("wh", [128, NKO, 2, 768], e4, kind="ExternalInput").ap()
    wl = nc.dram_tensor("wl", [128, NKO, 2, 768], e4, kind="ExternalInput").ap()
    woh = nc.dram_tensor("woh", [128, 2, 2, D], e4, kind="ExternalInput").ap()
    wol = nc.dram_tensor("wol", [128, 2, 2, D], e4, kind="ExternalInput").ap()
    cc = nc.dram_tensor("cc", [128, S], bf16, kind="ExternalInput").ap()
    ss = nc.dram_tensor("ss", [128, S], bf16, kind="ExternalInput").ap()
    maskd = nc.dram_tensor("maskd", [128, 1024], bf16, kind="ExternalInput").ap()
    onesd = nc.dram_tensor("onesd", [128, 1], bf16, kind="ExternalInput").ap()
    y = nc.dram_tensor("y", [S, D], bf16, kind="ExternalOutput").ap()

    with tile.TileContext(nc) as tc, ExitStack() as ctx:
        consts = ctx.enter_context(tc.tile_pool(name="consts", bufs=1))
        xp = ctx.enter_context(tc.tile_pool(name="xp", bufs=3))
        qk_pool = ctx.enter_context(tc.tile_pool(name="qk", bufs=1))
        v_pool = ctx.enter_context(tc.tile_pool(name="v", bufs=1))
        rope_pool = ctx.enter_context(tc.tile_pool(name="rope", bufs=3))
        vt_pool = ctx.enter_context(tc.tile_pool(name="vt", bufs=2))
        es_pool = ctx.enter_context(tc.tile_pool(name="es", bufs=4))
        acc_pool = ctx.enter_context(tc.tile_pool(name="accp", bufs=2))
        o_pool = ctx.enter_context(tc.tile_pool(name="o", bufs=2))
        r_pool = ctx.enter_context(tc.tile_pool(name="r", bufs=2))
        y_pool = ctx.enter_context(tc.tile_pool(name="y", bufs=4))

        wh_sb = consts.tile([128, NKO, 2, 768], e4)
        wl_sb = consts.tile([128, NKO, 2, 768], e4)
        woh_sb = consts.tile([128, 2, 2, D], e4)
        wol_sb = consts.tile([128, 2, 2, D], e4)
        cc_sb = consts.tile([128, S], bf16)
        ss_sb = consts.tile([128, S], bf16)
        mask_sb = consts.tile([128, 1024], bf16)
        ones_sb = consts.tile([128, 1], bf16)

        def load_w_chunk(ko, cols):
            whe = nc.sync if ko % 2 == 0 else nc.scalar
            wle = nc.scalar if ko % 2 == 0 else nc.sync
            whe.dma_start(wh_sb[:, ko, :, cols], wh[:, ko, :, cols])
            wle.dma_start(wl_sb[:, ko, :, cols], wl[:, ko, :, cols])

        def load_late_consts():
            nc.sync.dma_start(cc_sb[:], cc)
            nc.sync.dma_start(ss_sb[:], ss)
            nc.sync.dma_start(mask_sb[:], maskd)
            nc.sync.dma_start(onesd_t[:], onesd)

        onesd_t = ones_sb

        def load_woT():
            for ko in range(2):
                nc.sync.dma_start(woh_sb[:, ko, :, :], woh[:, ko, :, :])
                nc.sync.dma_start(wol_sb[:, ko, :, :], wol[:, ko, :, :])

        # persistent activations
        qkT = qk_pool.tile([128, 5, S], bf16)  # [e, {q0..q3,k}, s]
        v_sb = v_pool.tile([128, S // 128, 128], bf16)  # [s_in_blk, blk, e]

        # x chunk tiles. Prologue: per-ko DMAs on the (then-idle) gpsimd
        # queue so the first waves start early. Steady state: two big DMAs
        # on the SP/ACT hardware queues -- keeping them off Pool, whose
        # SWDGE descgen otherwise blocks the partition_broadcasts.
        def fetch_x(j, prologue=False):
            sj = slice(SC * j, SC * (j + 1))
            xh_t = xp.tile([128, NKO, 2, SC], e4, tag="xh")
            xl_t = xp.tile([128, NKO, 2, SC], e4, tag="xl")
            if prologue:
                for g in range(0, NKO, 2):
                    gs = slice(g, g + 2)
                    nc.gpsimd.dma_start(xh_t[:, gs, :, :], xh[:, gs, :, sj])
                for g in range(0, NKO, 2):
                    gs = slice(g, g + 2)
                    nc.gpsimd.dma_start(xl_t[:, gs, :, :], xl[:, gs, :, sj])
            else:
                nc.sync.dma_start(xh_t[:], xh[:, :, :, sj])
                nc.sync.dma_start(xl_t[:], xl[:, :, :, sj])
            return xh_t, xl_t

        with (
            tc.tile_pool(name="ps_mix", bufs=2, space="PSUM") as ps_mix,
            tc.tile_pool(name="ps_s", bufs=3, space="PSUM") as ps_sp,
            tc.tile_pool(name="ps_o", bufs=3, space="PSUM") as ps_op,
        ):

            def gen_proj_wave(j, xh_t, xl_t, wave, first=False):
                """One col-wave of chunk j's projection, as a generator of
                small PE quanta; q/k roped into qkT, v into v_sb (via DMA
                transpose)."""
                sj = slice(SC * j, SC * (j + 1))
                if True:
                    wsl = slice(128 * wave, 128 * (wave + 1))
                    ps_p = ps_mix.tile([128, SC], f32, tag="mix")
                    n = 0
                    # xh-consuming passes first: the xl DMA of a prefetched
                    # chunk may still be in flight when the wave starts
                    for ko in range(NKO):
                        if first and j == 0 and wave == 4:
                            load_w_chunk(ko, slice(0, 768))
                        for wt, xt in ((wh_sb, xh_t), (wl_sb, xh_t)):
                            nc.tensor.matmul(
                                ps_p[:],
                                wt[:, ko, :, wsl],
                                xt[:, ko, :, :],
                                start=(n == 0),
                                stop=False,
                                perf_mode=DR,
                            )
                            n += 1
                        yield
                    if first and j == 0 and wave == 4:
                        load_late_consts()
                    for ko in range(NKO):
                        nc.tensor.matmul(
                            ps_p[:],
                            wh_sb[:, ko, :, wsl],
                            xl_t[:, ko, :, :],
                            start=False,
                            stop=(ko == NKO - 1),
                            perf_mode=DR,
                        )
                        yield
                    if wave < 5:
                        # RoPE: drain -> swap (DMA) -> rotate (DVE, bf16 4x)
                        raw = rope_pool.tile([128, SC], bf16, tag="raw")
                        nc.scalar.mul(raw[:], ps_p[:], 1.0 / PSC)
                        sw = rope_pool.tile([128, SC], bf16, tag="sw")
                        nc.sync.dma_start(sw[0:64, :], raw[64:128, :])
                        nc.sync.dma_start(sw[64:128, :], raw[0:64, :])
                        t1 = rope_pool.tile([128, SC], bf16, tag="t1")
                        t2 = rope_pool.tile([128, SC], bf16, tag="t2")
                        nc.vector.tensor_mul(t1[:], raw[:], cc_sb[:, sj])
                        nc.vector.tensor_mul(t2[:], sw[:], ss_sb[:, sj])
                        nc.vector.tensor_sub(
                            qkT[0:64, wave, sj], t1[0:64, :], t2[0:64, :]
                        )
                        nc.vector.tensor_add(
                            qkT[64:128, wave, sj], t1[64:128, :], t2[64:128, :]
                        )
                    else:
                        vT = vt_pool.tile([128, SC], bf16, tag="vt")
                        nc.scalar.mul(vT[:], ps_p[:], 1.0 / PSC)
                        for b4 in range(4):
                            nc.sync.dma_start_transpose(
                                v_sb[:, 4 * j + b4, :],
                                vT[:, 128 * b4 : 128 * (b4 + 1)],
                            )

            def emit_wo_unit(m, fc, pool=None):
                """y[s-block m, f-chunk fc] = oT_m' @ wo (compensated fp8)."""
                fj = slice(SC * fc, SC * (fc + 1))
                mm = m % 4
                msl = slice(128 * mm, 128 * (mm + 1))
                oh_t, ol_t = o_sb[m // 4]
                ps_y = (pool or ps_mix).tile(
                    [128, SC], f32, tag="mix" if pool is None else "o")
                n = 0
                for ko in range(2):
                    ksl = slice(2 * ko, 2 * ko + 2)
                    for ot, wt in (
                        (oh_t, woh_sb),
                        (ol_t, woh_sb),
                        (oh_t, wol_sb),
                    ):
                        nc.tensor.matmul(
                            ps_y[:],
                            ot[:, ksl, msl],
                            wt[:, ko, :, fj],
                            start=(n == 0),
                            stop=(n == 5),
                            perf_mode=DR,
                        )
                        n += 1
                y_sb = y_pool.tile([128, SC], bf16)
                if (m * NCH + fc) % 3 == 0:
                    nc.scalar.copy(y_sb[:], ps_y[:])
                else:
                    nc.vector.tensor_copy(y_sb[:], ps_y[:])
                nc.sync.dma_start(y[128 * m : 128 * (m + 1), fj], y_sb[:])

            # o hi/lo tiles per chunk (oT*SO quantized), [p, head, s]
            o_sb = {}

            def drain(it, k):
                for _ in range(k):
                    try:
                        next(it)
                    except StopIteration:
                        return False
                return True

            def emit_attn_chunk(c, fill, nq):
                """Flat software-pipelined attention over all (h, jk) blocks
                of chunk c. `fill` is an iterator of small PE work quanta
                (wo units / proj-wave pieces), nq of them, spread evenly
                across blocks."""
                scj = slice(SC * c, SC * (c + 1))
                nblk = 4 * (c + 1)
                PD = 2

                def _lo(jk):
                    d = jk - 4 * c
                    return 0 if d < 1 else 128 * d

                blocks = [(h, jk) for h in range(HPC) for jk in range(nblk)]
                state = {}  # h -> (ps_o, acc)
                es_tiles = {}
                nb = len(blocks)
                for i in range(nb + PD):
                    if i < nb:
                        h, jk = blocks[i]
                        lo = _lo(jk)
                        if jk == 0:
                            ps_o = ps_op.tile([128, SC], f32, tag="o")
                            acc = acc_pool.tile([128, SC], bf16, tag="acc")
                            state[h] = (ps_o, acc)
                        ps_s = ps_sp.tile([128, SC], f32, tag="s")
                        nc.tensor.matmul(
                            ps_s[:, lo:SC],
                            qkT[:, 4, 128 * jk : 128 * (jk + 1)],
                            qkT[:, h, scj][:, lo:SC],
                            start=True,
                            stop=True,
                        )
                        es = es_pool.tile([128, SC], bf16, tag="es")
                        nc.scalar.activation(
                            es[:, lo:SC], ps_s[:, lo:SC], Exp, scale=SCALE
                        )
                        diag = jk - 4 * c
                        if diag >= 0:
                            off = 128 * diag
                            nc.vector.tensor_mul(
                                es[:, lo:SC],
                                es[:, lo:SC],
                                mask_sb[:, SC - off + lo : 2 * SC - off],
                            )
                        acc = state[h][1]
                        if jk == 0:
                            nc.vector.tensor_copy(acc[:], es[:])
                        else:
                            nc.vector.tensor_add(
                                acc[:, lo:SC], acc[:, lo:SC], es[:, lo:SC]
                            )
                        es_tiles[(h, jk)] = es
                    if i >= PD:
                        h, pj = blocks[i - PD]
                        es = es_tiles.pop((h, pj))
                        lo = _lo(pj)
                        ps_o = state[h][0]
                        nc.tensor.matmul(
                            ps_o[:, lo:SC],
                            v_sb[:, pj, :],
                            es[:, lo:SC],
                            start=pj == 0,
                            stop=pj == nblk - 1,
                        )
                        if pj == nblk - 1:
                            _finish_head(c, h, state.pop(h))
                    if i >= PD:
                        # back-loaded spread: ACT's exp backlog grows through
                        # the chunk, so give the PE more filler late
                        j0, j1 = i - PD, i + 1 - PD
                        drain(fill, j1**4 * nq // nb**4 - j0**4 * nq // nb**4)
                drain(fill, 1 << 30)

            def _finish_head(c, h, st):
                """Rowsum, normalize (ones=1/SO so rb=SO/rowsum), quantize."""
                ps_o, acc = st
                rs = ps_sp.tile([128, SC], f32, tag="s")
                nc.tensor.matmul(
                    rs[0:1, :],
                    ones_sb[:],
                    acc[:],
                    start=True,
                    stop=True,
                )
                r1 = r_pool.tile([1, SC], f32, tag="r1")
                nc.vector.reciprocal(r1[:], rs[0:1, :])
                rb = r_pool.tile([128, SC], f32, tag="rb")
                nc.gpsimd.partition_broadcast(rb[:], r1[:])
                oh_t, ol_t = o_sb[c]
                o8 = rope_pool.tile([128, SC], bf16, tag="o8")
                nc.vector.tensor_mul(o8[:], ps_o[:], rb[:])
                nc.scalar.copy(oh_t[:, h, :], o8[:])
                nc.vector.tensor_sub(ol_t[:, h, :], o8[:], oh_t[:, h, :])

            # ---------------- schedule ----------------
            # k-wave (4) first so the next chunk's scores unblock earliest
            WORDER = [4, 0, 1, 2, 3, 5]
            xh_t, xl_t = fetch_x(0, prologue=True)
            xns = {0: (xh_t, xl_t)}
            # prologue waves run in interleaved pairs: one wave consumes w
            # chunks faster than the two DMA queues supply them
            ga = gen_proj_wave(0, xh_t, xl_t, 4, first=True)
            gb = gen_proj_wave(0, xh_t, xl_t, 0)
            alive = [ga, gb]
            while alive:
                for g in list(alive):
                    try:
                        next(g)
                    except StopIteration:
                        alive.remove(g)
            xns[1] = fetch_x(1)
            for w in (1, 2, 3, 5):
                for _ in gen_proj_wave(0, xh_t, xl_t, w):
                    pass
            load_woT()

            def gen_wo_units(c, pool=None, alt=False):
                for h in range(HPC):
                    m = 4 * c + h
                    for fc in range(NCH):
                        p = pool if not (alt and fc % 2) else None
                        emit_wo_unit(m, fc, p)
                        yield

            def mix_fill(units, waves):
                """Interleave wo units between whole proj waves (both use the
                ps_mix pool, so units must not land inside an open wave)."""
                for wv in waves:
                    yield from wv
                for _ in units:
                    yield

            for c in range(NCH):
                oh_t = o_pool.tile([128, HPC, SC], e4, tag="oh")
                ol_t = o_pool.tile([128, HPC, SC], e4, tag="ol")
                o_sb[c] = (oh_t, ol_t)
                # wo units lag one extra chunk where possible: attn(3)
                # has no proj fills and would otherwise starve
                nq = 0
                units = iter(())
                waves = []
                if c + 2 < NCH:
                    xns[c + 2] = fetch_x(c + 2)
                if c == 1:
                    units = gen_wo_units(0)
                    nq += 16
                elif c == 3:
                    units = itertools.chain(gen_wo_units(1), gen_wo_units(2))
                    nq += 32
                if c + 1 < NCH:
                    waves = [gen_proj_wave(c + 1, *xns[c + 1], w) for w in WORDER]
                    nq += 96
                emit_attn_chunk(c, mix_fill(units, waves), nq)
            for _ in gen_wo_units(3, pool=ps_op):
                pass

    nc.compile()
    return nc


def _get_module():
    if "nc" not in _CACHE:
        _CACHE["nc"] = _build_module()
    return _CACHE["nc"]


def _pack_k(t):
    """[D_contr, n] -> [128, ko, 2, n] with d = 256*ko + 128*two + p."""
    dk = t.shape[0] // 256
    return np.ascontiguousarray(
        t.reshape(dk, 2, 128, *t.shape[1:]).transpose(2, 0, 1, 3)
    )


def _prep_inputs(x, freqs_cos, freqs_sin, wq, wk, wv, wo):
    """Host-side shard/layout/quantize prep. Returns per-core input maps."""
    import ml_dtypes

    E4 = ml_dtypes.float8_e4m3
    BF = ml_dtypes.bfloat16

    perm = np.concatenate([np.arange(0, 128, 2), np.arange(1, 128, 2)])
    x = np.asarray(x, np.float32)
    xT = x.transpose(0, 2, 1)  # [B, D, S]

    cosT = np.ascontiguousarray(np.asarray(freqs_cos, np.float32).T)  # [64,S]
    sinT = np.ascontiguousarray(np.asarray(freqs_sin, np.float32).T)
    cc = np.concatenate([cosT, cosT], 0).astype(BF)
    sst = np.concatenate([sinT, sinT], 0).astype(BF)
    # swapped-row multiplier: top rows get sin applied to p_i (rows 64:128
    # of raw land in rows 0:64 of sw) -> ss rows are just [s; s] too.
    p_idx = np.arange(128)[:, None]
    g_idx = np.arange(1024)[None, :]
    mask = (p_idx <= g_idx - 512).astype(np.float32).astype(BF)
    ones = np.full((128, 1), 1.0 / SO, np.float32).astype(BF)

    def hilo(t, s):
        th = (t * s).astype(E4)
        tl = ((t * s) - th.astype(np.float32)).astype(E4)
        return th, tl

    xb = {}
    for b in range(B):
        xbh, xbl = hilo(xT[b], SX)
        xb[b] = (_pack_k(xbh), _pack_k(xbl))

    in_maps = []
    for i in range(N_CORES):
        b, g = i // 4, i % 4
        wq_i = wq[512 * g : 512 * (g + 1)]  # [512, D]
        wq_i = np.concatenate([wq_i[128 * h + perm] for h in range(HPC)], 0)
        wk_i = wk[128 * g : 128 * (g + 1)][perm]
        wv_i = wv[128 * g : 128 * (g + 1)]
        w_i = np.ascontiguousarray(
            np.concatenate([wq_i, wk_i, wv_i], 0).T, dtype=np.float32
        )  # [D, 768]
        whq, wlq = hilo(w_i, SW)
        woT_i = np.ascontiguousarray(
            wo[:, 512 * g : 512 * (g + 1)].T, dtype=np.float32
        )  # [512, D]
        wohq, wolq = hilo(woT_i, SW)
        in_maps.append(
            {
                "xh": xb[b][0],
                "xl": xb[b][1],
                "wh": _pack_k(whq),
                "wl": _pack_k(wlq),
                "woh": _pack_k(wohq),
                "wol": _pack_k(wolq),
                "cc": cc,
                "ss": sst,
                "maskd": mask,
                "onesd": ones,
            }
        )
    return in_maps


def kernel(x, freqs_cos, freqs_sin, wq, wk, wv, wo):
    from concourse.bass_utils import run_bass_kernel_spmd

    nc = _get_module()
    in_maps = _prep_inputs(x, freqs_cos, freqs_sin, wq, wk, wv, wo)
    res = run_bass_kernel_spmd(nc, in_maps, list(range(N_CORES)))
    out = np.zeros((B, S, D), dtype=np.float32)
    for i in range(N_CORES):
        out[i // 4] += res.results[i]["y"].astype(np.float32)
    out *= 1.0 / (SO * SW)
    return out


if __name__ == "__main__":
    nc = _get_module()
    print(
        "instructions:",
        sum(len(blk.instructions) for blk in nc.m.functions[0].blocks),
    )

